# revision 1
# baseline (speedup 1.0000x reference)
"""GQA kernel for Trainium2, sharded over 8 NeuronCores.

Problem: B=2, S=2048, D=2048, H=16 q-heads, HKV=4 kv-heads, DH=128.
Sharding: core = b*4 + g handles batch b and kv-head group g (4 q-heads).
Each core computes its group's Q/K/V projections, attention, and the
row-sharded slice of the output projection; the host sums the 4 partial
outputs per batch (Wo row-parallel reduction).

Design (all matmuls bf16 — 1 PE cycle/row vs 4 for fp32; the 2e-2 rel-err
budget absorbs bf16 at ~9e-3 measured on hw; fp8 variants all exceed the
budget, verified by host emulation):
  - Inputs staged transposed ([D, S]) bf16; x chunks stream through SBUF
    in exact consumption order (serial DMA ~330GB/s is the projection-phase
    pacer, so weight loads are split/placed to never block the x stream).
  - Projections per s-block: Q (4 heads), K, V with weights stationary;
    q/k kept transposed [DH, S]; v^T transposed to natural [s, DH] tiles
    by a single XBAR DMA transpose (16x128 tiles, no PE work).
  - Attention per (head, q-block): scores^T = K @ Q^T per k-chunk into
    psum, exp on ACT -> P^T bf16, attn-out^T += V^T @ P^T on PE.
  - Softmax denominators entirely OFF the PE: DVE pairwise+running fp16
    sums of the P^T tiles (4x-mode eligible, ~0.1% error; running form
    keeps the post-last-exp critical chain short), one gpsimd
    partition_all_reduce (fp16 in, fp32 out) gives r broadcast across
    partitions, DVE reciprocal + multiply normalize (deferred division).
  - Scores+exp for (head 0, block 0) are emitted interleaved with the V
    projections (they only need kp/qp) so ACT — the early attention-phase
    pacer — gets a head start while the PE waits on the v DMA stream.
  - Out projection of block qb runs AFTER attention of qb+1 (one-block
    delay) so the PE fills the normalization-chain latency; bf16 output,
    host upcasts and sums the 4 partials per batch.
"""

import math
import sys

import numpy as np

if "/opt/trn_rl_repo" not in sys.path:
    sys.path.insert(0, "/opt/trn_rl_repo")

S = 2048
D = 2048
DH = 128
NH = 4  # q-heads per core (one GQA group)
DC = D // 128  # contraction chunks for projections
KC = S // 128  # k-chunks for attention
QB = 512  # q-block (matmul moving free dim)
NQB = S // QB
SCALE = 1.0 / math.sqrt(DH)
N_CORES = 8

LAST_EXEC_NS = None
LAST_RESULTS = None

_PROGRAM = None


def _emit(tc, nc, mybir, ReduceOp, qT, kT, vT, wq, wk, wv, wo, out):
    f32 = mybir.dt.float32
    bf16 = mybir.dt.bfloat16
    f16 = mybir.dt.float16
    Exp = mybir.ActivationFunctionType.Exp

    qT_r = qT[:].rearrange("(dc p) s -> p dc s", p=128)
    kT_r = kT[:].rearrange("(dc p) s -> p dc s", p=128)
    vT_r = vT[:].rearrange("(dc p) s -> p dc s", p=128)
    wq_r = wq[:].rearrange("(dc p) c -> p dc c", p=128)  # [128, DC, 512]
    wk_r = wk[:].rearrange("(dc p) c -> p dc c", p=128)  # [128, DC, 128]
    wv_r = wv[:].rearrange("(dc p) c -> p dc c", p=128)
    wo_r = wo[:].rearrange("(ck p) d -> p ck d", p=128)  # [128, NH, D]
    out_r = out[:].rearrange("(sc p) d -> p sc d", p=128)  # [128, S//128, D]

    with tc.tile_pool(name="persist", bufs=1) as persist, \
         tc.tile_pool(name="xstream", bufs=26) as xs_pool:
        wk_sb = persist.tile([128, DC, DH], bf16, tag="wk")
        wq_sb = persist.tile([128, DC, NH * DH], bf16, tag="wq")
        wv_sb = persist.tile([128, DC, DH], bf16, tag="wv")
        wo_sb = persist.tile([128, NH, D], bf16, tag="wo")
        kp = persist.tile([128, S], bf16, tag="kp")  # k_proj^T
        vp = persist.tile([128, KC, DH], bf16, tag="vp")  # v_proj natural
        vpT = persist.tile([128, S], bf16, tag="vpT")  # v_proj^T staging
        qp = persist.tile([128, NH, S], bf16, tag="qp")  # q_proj^T

        # x chunks stream in consumption order: [128, 2, QB] (a dc-pair for
        # one s block). Loaded just ahead of the matmuls that read them.
        def load_x(src_r, sb):
            tiles = []
            for j in range(DC // 2):
                xt = xs_pool.tile([128, 2, QB], bf16, tag="xs")
                nc.sync.dma_start(
                    out=xt,
                    in_=src_r[:, 2 * j:2 * j + 2, sb * QB:(sb + 1) * QB],
                )
                tiles.append(xt)
            return tiles

        # ---- schedule ----
        # DMA stream (serial ~330GB/s): wq||q0 interleaved, wk, k0-3, wv,
        # v0-3, then q1-3 trickle in while attention runs.  PE: Q0, K(all),
        # V(all); then per q-block: attention(qb) [+ Q-proj of qb+1 and
        # out-proj of qb-1 interleaved] so the attention phase hides the
        # rest of the input stream and the normalization latency.
        with tc.tile_pool(name="s_psum_outer", bufs=3, space="PSUM") as s_psum_o, \
             tc.tile_pool(name="pt_pool", bufs=26) as pt_pool, \
             tc.tile_pool(name="acc_pool", bufs=4) as acc_pool, \
             tc.tile_pool(name="avn_pool", bufs=2) as avn_pool, \
             tc.tile_pool(name="ostage", bufs=3) as ostage, \
             tc.tile_pool(name="tree_pool", bufs=10) as tree_pool:

            pj_box = {}
            psum_box = {"s": s_psum_o}

            def qproj(sb, qts):
                pj_psum = pj_box["pool"]
                for h in range(NH):
                    ps = pj_psum.tile([128, QB], f32, tag="pj")
                    for dc in range(DC):
                        nc.tensor.matmul(
                            ps,
                            lhsT=wq_sb[:, dc, h * DH:(h + 1) * DH],
                            rhs=qts[dc // 2][:, dc % 2, :],
                            start=(dc == 0), stop=(dc == DC - 1),
                        )
                    nc.scalar.copy(qp[:, h, sb * QB:(sb + 1) * QB], ps)

            # projections: per s-block Q (4 heads), K, V — x chunks stream
            # in the same order the PE consumes them.  Scores+exp for the
            # first attention head of block 0 are emitted interleaved here:
            # they only need kp/qp, fill the PE's V-phase DMA gaps, and give
            # the ACT engine (the attention-phase pacer) a ~25us head start.
            early = {"pts": [], "t1": []}

            def early_scores(kc_lo, kc_hi):
                s_psum = psum_box["s"]
                for kc in range(kc_lo, kc_hi):
                    ss = s_psum.tile([128, QB], f32, tag="s")
                    nc.tensor.matmul(
                        ss,
                        lhsT=kp[:, kc * 128:(kc + 1) * 128],
                        rhs=qp[:, 0, 0:QB],
                        start=True, stop=True,
                    )
                    pt = pt_pool.tile([128, QB], bf16, tag="pt")
                    nc.scalar.activation(pt, ss, Exp, scale=SCALE)
                    early["pts"].append(pt)
                    if kc % 2 == 1:
                        t = tree_pool.tile([128, QB], f16, tag="t1")
                        nc.vector.tensor_add(
                            t, early["pts"][kc - 1], early["pts"][kc])
                        if early["t1"]:
                            r_new = tree_pool.tile([128, QB], f16, tag="t2")
                            nc.vector.tensor_add(r_new, early["t1"][-1], t)
                            early["t1"].append(r_new)
                        else:
                            early["t1"].append(t)

            with tc.tile_pool(name="pj_psum", bufs=3, space="PSUM") as pj_psum, \
                 tc.tile_pool(name="vn_psum", bufs=2, space="PSUM") as vn_psum:
                pj_box["pool"] = pj_psum
                for sb in range(NQB):
                    qts = []
                    for j in range(DC // 2):
                        if sb == 0 and j < 4:
                            nc.sync.dma_start(out=wq_sb[:, 4 * j:4 * j + 4, :],
                                              in_=wq_r[:, 4 * j:4 * j + 4, :])
                        xt = xs_pool.tile([128, 2, QB], bf16, tag="xs")
                        nc.sync.dma_start(
                            out=xt,
                            in_=qT_r[:, 2 * j:2 * j + 2, sb * QB:(sb + 1) * QB])
                        qts.append(xt)
                    if sb == 0:
                        nc.sync.dma_start(out=wk_sb, in_=wk_r)
                    kts = load_x(kT_r, sb)
                    if sb == 0:
                        nc.sync.dma_start(out=wv_sb, in_=wv_r)
                    vts = load_x(vT_r, sb)
                    qproj(sb, qts)
                    ps = pj_psum.tile([128, QB], f32, tag="pj")
                    for dc in range(DC):
                        nc.tensor.matmul(
                            ps, lhsT=wk_sb[:, dc, :],
                            rhs=kts[dc // 2][:, dc % 2, :],
                            start=(dc == 0), stop=(dc == DC - 1),
                        )
                    nc.scalar.copy(kp[:, sb * QB:(sb + 1) * QB], ps)
                    early_scores(4 * sb, 4 * sb + 4)
                    psv = vn_psum.tile([128, QB], f32, tag="vn")
                    for dc in range(DC):
                        nc.tensor.matmul(
                            psv, lhsT=wv_sb[:, dc, :],
                            rhs=vts[dc // 2][:, dc % 2, :],
                            start=(dc == 0), stop=(dc == DC - 1),
                        )
                    nc.scalar.copy(vpT[:, sb * QB:(sb + 1) * QB], psv)
                    if sb == 2:
                        nc.sync.dma_start(out=wo_sb, in_=wo_r)
                nc.sync.dma_start(out=vp, in_=vpT, transpose=True)

            avns = [None] * NQB

            def attn(qb):
                s_psum = psum_box["s"]
                av_psum = psum_box["av"]
                qsl = slice(qb * QB, (qb + 1) * QB)
                avn_t = avn_pool.tile([128, NH, QB], bf16, tag="avn")
                avns[qb] = avn_t
                for h in range(NH):
                    av = av_psum.tile([128, QB], f32, tag="av")
                    if qb == 0 and h == 0:
                        for kc in range(KC):
                            nc.tensor.matmul(
                                av, lhsT=vp[:, kc, :], rhs=early["pts"][kc],
                                start=(kc == 0), stop=(kc == KC - 1),
                            )
                        t1 = early["t1"]
                        pts = early["pts"]
                    else:
                        t1 = []
                        pts = []
                    for kc in ([] if qb == 0 and h == 0 else range(KC)):
                        ss = s_psum.tile([128, QB], f32, tag="s")
                        nc.tensor.matmul(
                            ss,
                            lhsT=kp[:, kc * 128:(kc + 1) * 128],
                            rhs=qp[:, h, qsl],
                            start=True, stop=True,
                        )
                        pt = pt_pool.tile([128, QB], bf16, tag="pt")
                        nc.scalar.activation(pt, ss, Exp, scale=SCALE)
                        nc.tensor.matmul(
                            av, lhsT=vp[:, kc, :], rhs=pt,
                            start=(kc == 0), stop=(kc == KC - 1),
                        )
                        pts.append(pt)
                        if kc % 2 == 1:
                            t = tree_pool.tile([128, QB], f16, tag="t1")
                            nc.vector.tensor_add(t, pts[kc - 1], pts[kc])
                            if t1:
                                r_new = tree_pool.tile([128, QB], f16, tag="t2")
                                nc.vector.tensor_add(r_new, t1[-1], t)
                                t1.append(r_new)
                            else:
                                t1.append(t)
                    RR = acc_pool.tile([128, QB], f32, tag="RR")
                    nc.gpsimd.partition_all_reduce(
                        RR, t1[-1], channels=128, reduce_op=ReduceOp.add
                    )
                    rec = acc_pool.tile([128, QB], f32, tag="rec")
                    nc.vector.reciprocal(rec, RR)
                    nc.vector.tensor_mul(avn_t[:, h, :], av, rec)

            ot_box = {}

            def step_outproj(qb, step):
                # one (sc-row, db-column) accumulation of out = avn @ Wo_g
                o_psum = psum_box["o"]
                avn_t = avns[qb]
                j, db = step // 4, step % 4
                sc = qb * (QB // 128) + j
                if db == 0:
                    ot_new = ostage.tile([128, D], bf16, tag="ot")
                    ot_box[qb] = ot_new
                ot = ot_box[qb]
                po = o_psum.tile([128, QB], f32, tag="po")
                for ck in range(NH):
                    nc.tensor.matmul(
                        po,
                        lhsT=avn_t[:, ck, j * 128:(j + 1) * 128],
                        rhs=wo_sb[:, ck, db * QB:(db + 1) * QB],
                        start=(ck == 0), stop=(ck == NH - 1),
                    )
                if qb == NQB - 1:
                    if db % 2 == 0:
                        nc.scalar.copy(ot[:, db * QB:(db + 1) * QB], po)
                    else:
                        nc.vector.tensor_copy(ot[:, db * QB:(db + 1) * QB], po)
                    nc.sync.dma_start(
                        out=out_r[:, sc, db * QB:(db + 1) * QB],
                        in_=ot[:, db * QB:(db + 1) * QB])
                else:
                    nc.vector.tensor_copy(ot[:, db * QB:(db + 1) * QB], po)
                    if db == 3:
                        nc.sync.dma_start(out=out_r[:, sc, :], in_=ot)

            def outproj(qb):
                for step in range(16):
                    step_outproj(qb, step)

            with tc.tile_pool(name="av_psum", bufs=2, space="PSUM") as av_psum, \
                 tc.tile_pool(name="o_psum", bufs=3, space="PSUM") as o_psum:
                psum_box["av"] = av_psum
                psum_box["o"] = o_psum
                for qb in range(NQB):
                    attn(qb)
                    if qb >= 1:
                        outproj(qb - 1)
                outproj(NQB - 1)


def build_program():
    global _PROGRAM
    if _PROGRAM is not None:
        return _PROGRAM
    import concourse.tile as tile
    from concourse import bacc, mybir
    from concourse.bass_isa import ReduceOp

    bf16 = mybir.dt.bfloat16
    nc = bacc.Bacc("TRN2", target_bir_lowering=False, debug=False)
    qT = nc.declare_dram_parameter("qT", [D, S], bf16, isOutput=False)
    kT = nc.declare_dram_parameter("kT", [D, S], bf16, isOutput=False)
    vT = nc.declare_dram_parameter("vT", [D, S], bf16, isOutput=False)
    wq = nc.declare_dram_parameter("wq", [D, NH * DH], bf16, isOutput=False)
    wk = nc.declare_dram_parameter("wk", [D, DH], bf16, isOutput=False)
    wv = nc.declare_dram_parameter("wv", [D, DH], bf16, isOutput=False)
    wo = nc.declare_dram_parameter("wo", [NH * DH, D], bf16, isOutput=False)
    out = nc.declare_dram_parameter("out", [S, D], bf16, isOutput=True)

    with tile.TileContext(nc) as tc:
        _emit(tc, nc, mybir, ReduceOp, qT, kT, vT, wq, wk, wv, wo, out)

    nc.finalize()
    _PROGRAM = nc
    return nc


def make_in_maps(query, key, value, Wq, Wk, Wv, Wo):
    import ml_dtypes

    bf = ml_dtypes.bfloat16
    # transposed inputs shared across the 4 group-cores of each batch
    xTs = {}
    for b in range(2):
        xTs[b] = (
            np.ascontiguousarray(np.asarray(query[b], np.float32).T.astype(bf)),
            np.ascontiguousarray(np.asarray(key[b], np.float32).T.astype(bf)),
            np.ascontiguousarray(np.asarray(value[b], np.float32).T.astype(bf)),
        )
    in_maps = []
    for core in range(N_CORES):
        b, g = core // 4, core % 4
        qTb, kTb, vTb = xTs[b]
        in_maps.append({
            "qT": qTb,
            "kT": kTb,
            "vT": vTb,
            "wq": np.ascontiguousarray(np.asarray(Wq[:, g * 512:(g + 1) * 512], np.float32).astype(bf)),
            "wk": np.ascontiguousarray(np.asarray(Wk[:, g * 128:(g + 1) * 128], np.float32).astype(bf)),
            "wv": np.ascontiguousarray(np.asarray(Wv[:, g * 128:(g + 1) * 128], np.float32).astype(bf)),
            "wo": np.ascontiguousarray(np.asarray(Wo[g * 512:(g + 1) * 512, :], np.float32).astype(bf)),
        })
    return in_maps


def kernel(query, key, value, mask, Wq, Wk, Wv, Wo):
    global LAST_EXEC_NS, LAST_RESULTS
    del mask  # all-ones in this problem; softmax masking is a no-op
    nc = build_program()
    in_maps = make_in_maps(query, key, value, Wq, Wk, Wv, Wo)

    from concourse.bass_utils import run_bass_kernel_spmd

    res = run_bass_kernel_spmd(nc, in_maps, core_ids=list(range(N_CORES)))
    LAST_EXEC_NS = res.exec_time_ns
    LAST_RESULTS = res
    outs = [np.asarray(r["out"], np.float32) for r in res.results]
    full = np.empty((2, S, D), np.float32)
    for b in range(2):
        full[b] = outs[b * 4] + outs[b * 4 + 1] + outs[b * 4 + 2] + outs[b * 4 + 3]
    return full



# revision 33
# speedup vs baseline: 1.0578x; 1.0578x over previous
"""GQA kernel for Trainium2, sharded over 8 NeuronCores.

Problem: B=2, S=2048, D=2048, H=16 q-heads, HKV=4 kv-heads, DH=128.
Sharding: core = b*4 + g handles batch b and kv-head group g (4 q-heads).
Each core computes its group's Q/K/V projections, attention, and the
row-sharded slice of the output projection; the host sums the 4 partial
outputs per batch (Wo row-parallel reduction).

Design (split-fp8 DoubleRow for projections/out-proj, bf16 attention core):
  - Q/K/V projections and the out-projection run as fp8e4 DoubleRow
    matmuls (contraction 256/instr, 0.5 cycles/row -> 4x bf16 MAC rate)
    with each operand decomposed hi+lo at ONE shared scale
    (x: 16x, w: 512w): three accumulating passes hi*hi + lo*hi + hi*lo
    recover ~bf16 accuracy at 0.75x bf16 PE cost (measured ~9.4e-3 rel
    err vs 8.8e-3 all-bf16; single-fp8 operands fail the 2e-2 budget).
  - scores and attn*V stay bf16: P=exp(scores) in fp8 alone costs ~3e-2
    (e4m3 mantissa), and a P hi/lo split would need a second exp pass on
    the ACT engine, which is the #2-busiest engine (host-emulated).
  - exp uses bias=-2.5 via a memset const tile (softmax-invariant shift
    keeping exp outputs in fp8/fp16-safe range).
  - The PE queue is in-order, so emission order IS the schedule. The
    attention phase is ACT(exp)-paced, so every exp-wait is filled with
    DoubleRow matmuls pulled from generator "filler" streams
    (out-projection of the previous block, next block's q-projection,
    K/V projections during the prologue) via a gated FillCursor that
    tracks the DMA arrival order.
  - x tensors are host-packed tile-major fp8 (hi+lo in one DMA); weights
    are host-packed in their exact SBUF layouts; wq is split into
    head-pair halves so the first projection starts after only ~1.3MB
    of stream.
  - V projection is emitted in natural [kpos, dh] layout (x chunks as
    the stationary operand) so no transpose DMA is needed.
  - Softmax denominators off the PE: DVE pairwise sums of bf16 P tiles
    in fp16 (4x mode), one gpsimd partition_all_reduce, DVE reciprocal;
    avn = av * rec is split hi/lo fp8 on DVE for the DoubleRow
    out-projection.
  - PSUM (8 banks): big [128,1024]x2 (score pairs / dense q-proj),
    av [128,512]x2 (attention accumulators, also the prescored-head
    accumulators), o [128,512]x2 (out-proj accumulation, K/V projection
    accumulators in the prologue, filler q-proj).
"""

import math
import sys

import numpy as np

if "/opt/trn_rl_repo" not in sys.path:
    sys.path.insert(0, "/opt/trn_rl_repo")

S = 2048
D = 2048
DH = 128
NH = 4  # q-heads per core (one GQA group)
DC = 16  # contraction 128-chunks for projections
NJP = DC // 2  # dc-pairs per DoubleRow pass
KC = S // 128  # k-chunks for attention
NKP = KC // 2  # k-chunk pairs per (head, q-block)
QB = 512  # q-block
NQB = S // QB
SCALE = 1.0 / math.sqrt(DH)
EXP_BIAS = -2.5
X_SCALE = 16.0
W_SCALE = 512.0
PSUM_SCALE = X_SCALE * W_SCALE  # 8192: q/k/v (and out) psum scale
V_KEEP = X_SCALE / PSUM_SCALE  # vp kept at 16*v so avn lands at 16*avn
N_CORES = 8

LAST_EXEC_NS = None
LAST_RESULTS = None

_PROGRAM = None

_DONE = object()


class FillCursor:
    """Pulls matmul-emitting generator items with position tracking so
    the schedule can gate on known stream positions. ``limit`` caps how
    far opportunistic pulls may advance (hard data dependencies that the
    chain order alone cannot express)."""

    def __init__(self, gen):
        self.gen = gen
        self.pos = 0
        self.done = False
        self.limit = 1 << 30

    def pull(self, n):
        self.pull_to(self.pos + n)

    def pull_to(self, target):
        target = min(target, self.limit)
        while self.pos < target and not self.done:
            if next(self.gen, _DONE) is _DONE:
                self.done = True
            else:
                self.pos += 1

    def drain(self):
        self.limit = 1 << 30
        self.pull_to(1 << 30)


def _chain(*gens):
    for g in gens:
        yield from g


def _emit(tc, nc, mybir, ReduceOp, xq, xk, xv, wq01, wq23, wk, wv, wo, out):
    f32 = mybir.dt.float32
    bf16 = mybir.dt.bfloat16
    f16 = mybir.dt.float16
    f8 = mybir.dt.float8e4
    Exp = mybir.ActivationFunctionType.Exp
    DR = mybir.MatmulPerfMode.DoubleRow

    # x tensors are host-packed tile-major: [(sb j p), 2*2*QB]
    xq_r = xq[:].rearrange("(sb j p) w -> p sb j w", sb=NQB, j=NJP)
    xk_r = xk[:].rearrange("(sb j p) w -> p sb j w", sb=NQB, j=NJP)
    xv_r = xv[:].rearrange("(sb j p) w -> p sb j w", sb=NQB, j=NJP)
    out_r = out[:].rearrange("(sc p) d -> p sc d", p=128)

    with tc.tile_pool(name="persist", bufs=1) as persist, \
         tc.tile_pool(name="xstream", bufs=21) as xs_pool:
        wq01_sb = persist.tile([128, DC, 2, 2 * DH], f8, tag="wq01")
        wq23_sb = persist.tile([128, DC, 2, 2 * DH], f8, tag="wq23")
        wk_sb = persist.tile([128, DC, 2, DH], f8, tag="wk")
        wv_sb = persist.tile([128, DC, 2, DH], f8, tag="wv")
        wo_sb = persist.tile([128, NH, 2, D], f8, tag="wo")
        kp = persist.tile([128, S], bf16, tag="kp")  # k_proj^T (true scale)
        qp = persist.tile([128, NH, S], bf16, tag="qp")  # q_proj^T
        vp = persist.tile([128, KC, DH], bf16, tag="vp")  # v natural, 16*v
        ebias = persist.tile([128, 1], f32, tag="ebias")  # exp bias const

        def wq_slice(h, j, w_hl):
            t = wq01_sb if h < 2 else wq23_sb
            hh = h % 2
            return t[:, 2 * j:2 * j + 2, w_hl, hh * DH:(hh + 1) * DH]

        def load_x(src_r, sb):
            # one tile per dc-pair carrying hi+lo: [128, 2(pair), 2(hl), QB]
            tiles = []
            for j in range(NJP):
                xt = xs_pool.tile([128, 2, 2, QB], f8, tag="xs")
                nc.sync.dma_start(out=xt, in_=src_r[:, sb, j, :])
                tiles.append(xt)
            return tiles

        with tc.tile_pool(name="big_psum", bufs=2, space="PSUM") as big_psum, \
             tc.tile_pool(name="pt_pool", bufs=16) as pt_pool, \
             tc.tile_pool(name="pt2_pool", bufs=6) as pt2_pool, \
             tc.tile_pool(name="tree_pool", bufs=5) as tree_pool, \
             tc.tile_pool(name="acc_pool", bufs=2) as acc_pool, \
             tc.tile_pool(name="avn_pool", bufs=2) as avn_pool, \
             tc.tile_pool(name="ostage", bufs=2) as ostage:

            # (weights-hl, x-hl) per split pass: hi*hi, lo*hi, hi*lo
            PASS_HL = ((0, 0), (1, 0), (0, 1))

            nc.gpsimd.memset(ebias, EXP_BIAS)

            def gen_kproj(sb, kts):
                # j-outer: consume each x tile fully as it lands; o-pool psum
                ps = o_box["pool"].tile([128, QB], f32, tag="o", name="psk")
                n = 0
                for j in range(NJP):
                    for (w_hl, x_hl) in PASS_HL:
                        for half in range(2):
                            nc.tensor.matmul(
                                ps[:, half * 256:half * 256 + 256],
                                lhsT=wk_sb[:, 2 * j:2 * j + 2, w_hl, :],
                                rhs=kts[j][:, :, x_hl,
                                           half * 256:half * 256 + 256],
                                start=(n == 0), stop=(n == 47),
                                perf_mode=DR,
                            )
                            n += 1
                            if n == 48:
                                nc.scalar.mul(kp[:, sb * QB:(sb + 1) * QB],
                                              ps, 1.0 / PSUM_SCALE)
                            yield

            def gen_vproj(sb, vts):
                # natural layout: x pairs stationary, out [kpos, dh]; j-outer
                ps = o_box["pool"].tile([128, QB], f32, tag="o", name="psv")
                n = 0
                for j in range(NJP):
                    for kb in range(4):
                        for (w_hl, x_hl) in PASS_HL:
                            nc.tensor.matmul(
                                ps[:, kb * 128:(kb + 1) * 128],
                                lhsT=vts[j][:, :, x_hl,
                                            kb * 128:(kb + 1) * 128],
                                rhs=wv_sb[:, 2 * j:2 * j + 2, w_hl, :],
                                start=(n == 0), stop=(n == 95),
                                perf_mode=DR,
                            )
                            n += 1
                            if n == 96:
                                nc.scalar.mul(
                                    vp[:, 4 * sb:4 * sb + 4, :],
                                    ps[:].rearrange("p (a b) -> p a b", a=4),
                                    V_KEEP)
                            yield

            def qproj2(sb, h0, qts, act_copy=True):
                # heads h0, h0+1 into one [128,1024] psum (one bank each)
                ps = big_psum.tile([128, 1024], f32, tag="big")
                n = 0
                for j in range(NJP):
                    for hh in range(2):
                        for (w_hl, x_hl) in PASS_HL:
                            for half in range(2):
                                off = hh * 512 + half * 256
                                nc.tensor.matmul(
                                    ps[:, off:off + 256],
                                    lhsT=wq_slice(h0 + hh, j, w_hl),
                                    rhs=qts[j][:, :, x_hl,
                                               half * 256:half * 256 + 256],
                                    start=(n == 6 * hh), stop=(n == 89 + 6 * hh),
                                    perf_mode=DR,
                                )
                                n += 1
                dst = qp[:, h0:h0 + 2, sb * QB:(sb + 1) * QB]
                srcv = ps[:].rearrange("p (a b) -> p a b", a=2)
                if act_copy:
                    nc.scalar.mul(dst, srcv, 1.0 / PSUM_SCALE)
                else:
                    nc.vector.tensor_scalar_mul(dst, srcv, 1.0 / PSUM_SCALE)

            def gen_qproj_opsum(sb, h0, qts):
                # q-projection as a filler stream on the o-psum pool
                for hh in range(2):
                    h = h0 + hh
                    o = o_box["pool"].tile([128, QB], f32, tag="o", name="oq")
                    n = 0
                    for j in range(NJP):
                        for (w_hl, x_hl) in PASS_HL:
                            for half in range(2):
                                nc.tensor.matmul(
                                    o[:, half * 256:half * 256 + 256],
                                    lhsT=wq_slice(h, j, w_hl),
                                    rhs=qts[j][:, :, x_hl,
                                               half * 256:half * 256 + 256],
                                    start=(n == 0), stop=(n == 47),
                                    perf_mode=DR,
                                )
                                n += 1
                                if n == 48:
                                    nc.vector.tensor_scalar_mul(
                                        qp[:, h, sb * QB:(sb + 1) * QB], o,
                                        1.0 / PSUM_SCALE)
                                yield

            avns = [None] * NQB  # (avn_h, avn_l) per q-block

            def new_avns(qb):
                avns[qb] = (
                    avn_pool.tile([128, NH, QB], f8, tag="avh", name="avh"),
                    avn_pool.tile([128, NH, QB], f8, tag="avl", name="avl"))

            def score_pair(h, qb, kcp, state, pool=None):
                # scores+exp for k-chunk pair kcp; DVE running sums
                ss = big_psum.tile([128, 1024], f32, tag="big")
                for i in range(2):
                    kc = 2 * kcp + i
                    nc.tensor.matmul(
                        ss[:, i * 512:(i + 1) * 512],
                        lhsT=kp[:, kc * 128:(kc + 1) * 128],
                        rhs=qp[:, h, qb * QB:(qb + 1) * QB],
                        start=True, stop=True,
                    )
                pt = (pool or pt2_pool).tile([128, 2, QB], bf16, tag="pt",
                                             name="pt")
                nc.scalar.activation(
                    pt[:].rearrange("p a b -> p (a b)"), ss, Exp,
                    scale=SCALE, bias=ebias[:, 0:1])
                state["pts"].append(pt)
                if kcp % 2 == 1:
                    t = tree_pool.tile([128, 2, QB], f16, tag="t1")
                    nc.vector.tensor_add(t, state["pts"][kcp - 1],
                                         state["pts"][kcp])
                    if state["run"] is None:
                        state["run"] = t
                    else:
                        r_new = tree_pool.tile([128, 2, QB], f16, tag="t2")
                        nc.vector.tensor_add(r_new, state["run"], t)
                        state["run"] = r_new

            def av_pair(kcp, state):
                av, pt = state["av"], state["pts"][kcp]
                for i in range(2):
                    kc = 2 * kcp + i
                    nc.tensor.matmul(
                        av, lhsT=vp[:, kc, :], rhs=pt[:, i, :],
                        start=(kc == 0), stop=(kc == KC - 1),
                    )

            def finish_head(h, qb, state):
                # softmax denominators + avn hi/lo, entirely on DVE/Pool
                av = state["av"]
                sfin = tree_pool.tile([128, QB], f16, tag="tf")
                nc.vector.tensor_add(sfin, state["run"][:, 0, :],
                                     state["run"][:, 1, :])
                RR = acc_pool.tile([128, QB], f32, tag="RR")
                nc.gpsimd.partition_all_reduce(
                    RR, sfin, channels=128, reduce_op=ReduceOp.add)
                rec = acc_pool.tile([128, QB], f32, tag="rec")
                nc.vector.reciprocal(rec, RR)
                tmp = acc_pool.tile([128, QB], f32, tag="tmp")
                nc.vector.tensor_mul(tmp, av, rec)
                avn_h, avn_l = avns[qb]
                nc.vector.tensor_copy(avn_h[:, h, :], tmp)
                nc.vector.tensor_sub(avn_l[:, h, :], tmp, avn_h[:, h, :])

            def gen_av_seg(states, kcp_lo, kcp_hi, last=False):
                # AV filler for prescored heads 0/1, kcp in [lo, hi)
                for kcp in range(kcp_lo, kcp_hi):
                    for h in (0, 1):
                        st = states[h]
                        for i in range(2):
                            kc = 2 * kcp + i
                            nc.tensor.matmul(
                                st["av"], lhsT=vp[:, kc, :],
                                rhs=st["pts"][kcp][:, i, :],
                                start=(kc == 0), stop=(kc == KC - 1),
                            )
                            if (last and kcp == kcp_hi - 1 and h == 1
                                    and i == 1):
                                finish_head(0, 0, states[0])
                                finish_head(1, 0, states[1])
                            yield

            def run_duo(qb, hA, hB, fill, av_gate=None, p_ss=4, p_av=2):
                # lag-2 interleave of two heads; filler mms cover exp waits
                sts = {}
                for h in (hA, hB):
                    st = {"pts": [], "run": None}
                    st["av"] = av_box["pool"].tile([128, QB], f32, tag="av",
                                                   name="av")
                    sts[h] = st

                def do_av(kcp):
                    if av_gate is not None:
                        fill.pull_to(av_gate[kcp])
                    av_pair(kcp, sts[hA])
                    fill.pull(p_av)
                    av_pair(kcp, sts[hB])
                    fill.pull(p_av)

                for kcp in range(NKP):
                    score_pair(hA, qb, kcp, sts[hA])
                    fill.pull(p_ss)
                    score_pair(hB, qb, kcp, sts[hB])
                    fill.pull(p_ss)
                    if kcp >= 2:
                        do_av(kcp - 2)
                for kcp in (NKP - 2, NKP - 1):
                    fill.pull(p_ss)
                    do_av(kcp)
                finish_head(hA, qb, sts[hA])
                finish_head(hB, qb, sts[hB])
                fill.pull(2 * p_ss)

            def gen_outproj(qb):
                # out-projection of block qb as a filler stream (o-psum)
                avn_h, avn_l = avns[qb]
                for j in range(4):
                    jsl = slice(j * 128, (j + 1) * 128)
                    sc = qb * (QB // 128) + j
                    ot = ostage.tile([128, D], bf16, tag="ot", name="ot")
                    for dbp in range(4):
                        o = o_box["pool"].tile([128, 512], f32, tag="o",
                                               name="o")
                        n = 0
                        for (a_t, w_hl) in ((avn_h, 0), (avn_h, 1),
                                            (avn_l, 0)):
                            for db01 in range(2):
                                db = dbp * 2 + db01
                                for cp in range(2):
                                    nc.tensor.matmul(
                                        o[:, db01 * 256:db01 * 256 + 256],
                                        lhsT=a_t[:, 2 * cp:2 * cp + 2, jsl],
                                        rhs=wo_sb[:, 2 * cp:2 * cp + 2, w_hl,
                                                  db * 256:(db + 1) * 256],
                                        start=(n == 0), stop=(n == 11),
                                        perf_mode=DR,
                                    )
                                    n += 1
                                    if n == 12:
                                        osl = slice(dbp * 512,
                                                    (dbp + 1) * 512)
                                        nc.vector.tensor_scalar_mul(
                                            ot[:, osl], o, 1.0 / PSUM_SCALE)
                                        if dbp == 3:
                                            nc.sync.dma_start(
                                                out=out_r[:, sc, :], in_=ot)
                                    yield

            def gen_outproj_big(qb):
                # final out-projection on the (then idle) big psum pool:
                # [128,1024] tiles hold 2 db-pairs -> 4-bank pipelining
                avn_h, avn_l = avns[qb]
                for j in range(4):
                    jsl = slice(j * 128, (j + 1) * 128)
                    sc = qb * (QB // 128) + j
                    ot = ostage.tile([128, D], bf16, tag="ot", name="ot")
                    for dbq in range(2):
                        o = big_psum.tile([128, 1024], f32, tag="big",
                                          name="obig")
                        for half in range(2):
                            dbp = dbq * 2 + half
                            n = 0
                            for (a_t, w_hl) in ((avn_h, 0), (avn_h, 1),
                                                (avn_l, 0)):
                                for db01 in range(2):
                                    db = dbp * 2 + db01
                                    for cp in range(2):
                                        nc.tensor.matmul(
                                            o[:, half * 512 + db01 * 256:
                                              half * 512 + db01 * 256 + 256],
                                            lhsT=a_t[:, 2 * cp:2 * cp + 2,
                                                     jsl],
                                            rhs=wo_sb[:, 2 * cp:2 * cp + 2,
                                                      w_hl,
                                                      db * 256:(db + 1) * 256],
                                            start=(n == 0), stop=(n == 11),
                                            perf_mode=DR,
                                        )
                                        n += 1
                                        if n == 12 and half == 1:
                                            osl = slice(dbq * 1024,
                                                        (dbq + 1) * 1024)
                                            srcv = o[:].rearrange(
                                                "p (a b) -> p a b", a=2)
                                            dstv = ot[:, osl].rearrange(
                                                "p (a b) -> p a b", a=2)
                                            if dbq == 0:
                                                nc.vector.tensor_scalar_mul(
                                                    dstv, srcv,
                                                    1.0 / PSUM_SCALE)
                                            else:
                                                nc.scalar.mul(
                                                    dstv, srcv,
                                                    1.0 / PSUM_SCALE)
                                            if dbq == 1:
                                                nc.sync.dma_start(
                                                    out=out_r[:, sc, :],
                                                    in_=ot)
                                        yield

            av_box = {}
            o_box = {}
            with tc.tile_pool(name="av_psum", bufs=2, space="PSUM") as av_ps, \
                 tc.tile_pool(name="o_psum", bufs=2, space="PSUM") as o_ps:
                av_box["pool"] = av_ps
                o_box["pool"] = o_ps

                # ---- DMA stream order (arrival order = gate order) ----
                nc.sync.dma_start(out=wq01_sb, in_=wq01[:])
                qts0 = load_x(xq_r, 0)
                nc.sync.dma_start(out=wq23_sb, in_=wq23[:])
                nc.sync.dma_start(out=wk_sb, in_=wk[:])
                kts = [load_x(xk_r, sb) for sb in range(NQB)]
                nc.sync.dma_start(out=wv_sb, in_=wv[:])
                vts = [load_x(xv_r, sb) for sb in range(NQB)]
                qts_rest = [load_x(xq_r, 1)]
                nc.sync.dma_start(out=wo_sb, in_=wo[:])
                qts_rest += [load_x(xq_r, sb) for sb in range(2, NQB)]

                # ---- PE schedule ----
                qproj2(0, 0, qts0)
                qproj2(0, 2, qts0)

                new_avns(0)
                early = {h: {"pts": [], "run": None} for h in range(2)}
                for h in range(2):
                    early[h]["av"] = av_ps.tile([128, QB], f32, tag="av",
                                                name="eav")

                # fill chain in DMA arrival order; positions:
                # k0:48 k1:96 k2:144 k3:192 | v0:288 av01:296 v1:392 av12:400
                # v2:496 av23:504 v3:600 av34:608 av46:624 av68(+fin):640
                # qp1a:688 qp1b:736
                fill = FillCursor(_chain(
                    gen_kproj(0, kts[0]), gen_kproj(1, kts[1]),
                    gen_kproj(2, kts[2]), gen_kproj(3, kts[3]),
                    gen_vproj(0, vts[0]), gen_av_seg(early, 0, 1),
                    gen_vproj(1, vts[1]), gen_av_seg(early, 1, 2),
                    gen_vproj(2, vts[2]), gen_av_seg(early, 2, 3),
                    gen_vproj(3, vts[3]), gen_av_seg(early, 3, 4),
                    gen_av_seg(early, 4, 6),
                    gen_av_seg(early, 6, 8, last=True),
                    gen_qproj_opsum(1, 0, qts_rest[0]),
                    gen_qproj_opsum(1, 2, qts_rest[0]),
                ))
                KEND = {0: 48, 1: 96, 2: 144, 3: 192}

                # prescore qb0 heads 0/1, exp-paced, filled from the chain
                for kcp in range(NKP):
                    fill.pull_to(KEND[kcp // 2])
                    for h in range(2):
                        score_pair(h, 0, kcp, early[h], pool=pt_pool)
                        fill.pull(12)

                # qb0 h2/h3 duo; gates pull the chain so vp(sb) and the
                # early heads' remaining AV pairs land before use
                AV_GATE0 = {0: 296, 1: 296, 2: 400, 3: 400,
                            4: 504, 5: 504, 6: 640, 7: 640}
                run_duo(0, 2, 3, fill, av_gate=AV_GATE0)
                fill.drain()

                # steady state: one global fill chain across qb1-3.
                # positions: op0:192 qp2a:288 qp2b:384 | op1:576 qp3a:672
                # qp3b:768 | op2:960
                new_avns(1)
                gfill = FillCursor(_chain(
                    gen_outproj(0),
                    gen_qproj_opsum(2, 0, qts_rest[1]),
                    gen_qproj_opsum(2, 2, qts_rest[1]),
                    gen_outproj(1),
                    gen_qproj_opsum(3, 0, qts_rest[2]),
                    gen_qproj_opsum(3, 2, qts_rest[2]),
                    gen_outproj(2),
                ))
                # during qb's duos, pulls must stop before gen_outproj(qb)
                # (its avn tiles are only complete once qb finishes)
                QGATE = {2: 384, 3: 768}
                LIMIT = {1: 384, 2: 768, 3: 1 << 30}
                for qb in range(1, NQB):
                    gfill.limit = LIMIT[qb]
                    if qb > 1:
                        new_avns(qb)
                        gfill.pull_to(QGATE[qb])
                    run_duo(qb, 0, 1, gfill, p_ss=10, p_av=5)
                    run_duo(qb, 2, 3, gfill, p_ss=10, p_av=5)
                gfill.drain()
                FillCursor(gen_outproj_big(NQB - 1)).drain()


def build_program():
    global _PROGRAM
    if _PROGRAM is not None:
        return _PROGRAM
    import concourse.tile as tile
    from concourse import bacc, mybir
    from concourse.bass_isa import ReduceOp

    f8 = mybir.dt.float8e4
    bf16 = mybir.dt.bfloat16
    nc = bacc.Bacc("TRN2", target_bir_lowering=False, debug=False)
    # x: tile-major [(sb j p), 2*2*QB]; w: sbuf layout [128, dc*2*cols]
    xq = nc.declare_dram_parameter("xq", [NQB * NJP * 128, 4 * QB], f8,
                                   isOutput=False)
    xk = nc.declare_dram_parameter("xk", [NQB * NJP * 128, 4 * QB], f8,
                                   isOutput=False)
    xv = nc.declare_dram_parameter("xv", [NQB * NJP * 128, 4 * QB], f8,
                                   isOutput=False)
    wq01 = nc.declare_dram_parameter("wq01", [128, DC * 2 * 2 * DH], f8,
                                     isOutput=False)
    wq23 = nc.declare_dram_parameter("wq23", [128, DC * 2 * 2 * DH], f8,
                                     isOutput=False)
    wk = nc.declare_dram_parameter("wk", [128, DC * 2 * DH], f8,
                                   isOutput=False)
    wv = nc.declare_dram_parameter("wv", [128, DC * 2 * DH], f8,
                                   isOutput=False)
    wo = nc.declare_dram_parameter("wo", [128, NH * 2 * D], f8,
                                   isOutput=False)
    out = nc.declare_dram_parameter("out", [S, D], bf16, isOutput=True)

    with tile.TileContext(nc) as tc:
        _emit(tc, nc, mybir, ReduceOp, xq, xk, xv, wq01, wq23, wk, wv, wo,
              out)

    nc.finalize()
    _PROGRAM = nc
    return nc


def _split8(x, scale):
    """hi+lo e4m3 pair at one shared scale, stacked on a new axis 1."""
    import ml_dtypes

    f8 = ml_dtypes.float8_e4m3
    xs = np.asarray(x, np.float32) * scale
    hi = xs.astype(f8)
    lo = (xs - hi.astype(np.float32)).astype(f8)
    return np.stack([hi, lo], axis=1)  # [d0, 2, d1]


def _pack_x(xT):
    """[D, S] f32 -> tile-major fp8 [(sb j p), 2*2*QB].

    Tile (sb, j) holds element [p, i, hl, s] = split(xT)[(2j+i)*128+p,
    hl, sb*QB+s]."""
    sp = _split8(xT, X_SCALE)  # [D, 2, S]
    t = sp.reshape(NJP, 2, 128, 2, NQB, QB).transpose(4, 0, 2, 1, 3, 5)
    return np.ascontiguousarray(t.reshape(NQB * NJP * 128, 4 * QB))


def _pack_w(w):
    """[D, C] f32 -> sbuf-layout fp8 [128, (dc 2 C)]."""
    sp = _split8(w, W_SCALE)  # [D, 2, C]
    c = sp.shape[2]
    t = sp.reshape(-1, 128, 2, c).transpose(1, 0, 2, 3)
    return np.ascontiguousarray(t.reshape(128, -1))


def make_in_maps(query, key, value, Wq, Wk, Wv, Wo):
    xTs = {}
    for b in range(2):
        xTs[b] = (
            _pack_x(np.asarray(query[b], np.float32).T),
            _pack_x(np.asarray(key[b], np.float32).T),
            _pack_x(np.asarray(value[b], np.float32).T),
        )
    in_maps = []
    for core in range(N_CORES):
        b, g = core // 4, core % 4
        xqb, xkb, xvb = xTs[b]
        Wqg = np.asarray(Wq[:, g * 512:(g + 1) * 512], np.float32)
        in_maps.append({
            "xq": xqb,
            "xk": xkb,
            "xv": xvb,
            "wq01": _pack_w(Wqg[:, 0:256]),
            "wq23": _pack_w(Wqg[:, 256:512]),
            "wk": _pack_w(np.asarray(Wk[:, g * 128:(g + 1) * 128], np.float32)),
            "wv": _pack_w(np.asarray(Wv[:, g * 128:(g + 1) * 128], np.float32)),
            "wo": _pack_w(np.asarray(Wo[g * 512:(g + 1) * 512, :], np.float32)),
        })
    return in_maps


def kernel(query, key, value, mask, Wq, Wk, Wv, Wo):
    global LAST_EXEC_NS, LAST_RESULTS
    del mask  # all-ones in this problem; softmax masking is a no-op
    nc = build_program()
    in_maps = make_in_maps(query, key, value, Wq, Wk, Wv, Wo)

    from concourse.bass_utils import run_bass_kernel_spmd

    res = run_bass_kernel_spmd(nc, in_maps, core_ids=list(range(N_CORES)))
    LAST_EXEC_NS = res.exec_time_ns
    LAST_RESULTS = res
    outs = [np.asarray(r["out"], np.float32) for r in res.results]
    full = np.empty((2, S, D), np.float32)
    for b in range(2):
        full[b] = outs[b * 4] + outs[b * 4 + 1] + outs[b * 4 + 2] + outs[b * 4 + 3]
    return full


# revision 41
# speedup vs baseline: 1.0892x; 1.0297x over previous
"""GQA kernel for Trainium2, sharded over 8 NeuronCores.

Problem: B=2, S=2048, D=2048, H=16 q-heads, HKV=4 kv-heads, DH=128.
Sharding: core = b*4 + g handles batch b and kv-head group g (4 q-heads).
Each core computes its group's Q/K/V projections, attention, and the
row-sharded slice of the output projection; the host sums the 4 partial
outputs per batch (Wo row-parallel reduction).

Design (split-fp8 DoubleRow for projections/out-proj, bf16 attention core):
  - Q/K/V projections and the out-projection run as fp8e4 DoubleRow
    matmuls (contraction 256/instr, 0.5 cycles/row -> 4x bf16 MAC rate)
    with each operand decomposed hi+lo at ONE shared scale
    (x: 16x, w: 512w): three accumulating passes hi*hi + lo*hi + hi*lo
    recover ~bf16 accuracy at 0.75x bf16 PE cost (measured ~9.4e-3 rel
    err vs 8.8e-3 all-bf16; single-fp8 operands fail the 2e-2 budget).
  - scores and attn*V stay bf16: P=exp(scores) in fp8 alone costs ~3e-2
    (e4m3 mantissa), and a P hi/lo split would need a second exp pass on
    the ACT engine, which is the #2-busiest engine (host-emulated).
  - exp uses bias=-2.5 via a memset const tile (softmax-invariant shift
    keeping exp outputs in fp8/fp16-safe range).
  - The PE queue is in-order, so emission order IS the schedule. The
    attention phase is ACT(exp)-paced, so every exp-wait is filled with
    DoubleRow matmuls pulled from generator "filler" streams
    (out-projection of the previous block, next block's q-projection,
    K/V projections during the prologue) via a gated FillCursor that
    tracks the DMA arrival order.
  - x tensors are host-packed tile-major fp8 (hi+lo in one DMA); weights
    are host-packed in their exact SBUF layouts; wq is split into
    head-pair halves so the first projection starts after only ~1.3MB
    of stream.
  - V projection is emitted in natural [kpos, dh] layout (x chunks as
    the stationary operand) so no transpose DMA is needed.
  - Softmax denominators off the PE: DVE pairwise sums of bf16 P tiles
    in fp16 (4x mode), one gpsimd partition_all_reduce, DVE reciprocal;
    avn = av * rec is split hi/lo fp8 on DVE for the DoubleRow
    out-projection.
  - PSUM (8 banks): big [128,1024]x2 (score pairs / dense q-proj),
    av [128,512]x2 (attention accumulators, also the prescored-head
    accumulators), o [128,512]x2 (out-proj accumulation, K/V projection
    accumulators in the prologue, filler q-proj).
"""

import math
import sys

import numpy as np

if "/opt/trn_rl_repo" not in sys.path:
    sys.path.insert(0, "/opt/trn_rl_repo")

S = 2048
D = 2048
DH = 128
NH = 4  # q-heads per core (one GQA group)
DC = 16  # contraction 128-chunks for projections
NJP = DC // 2  # dc-pairs per DoubleRow pass
KC = S // 128  # k-chunks for attention
NKP = KC // 2  # k-chunk pairs per (head, q-block)
QB = 512  # q-block
NQB = S // QB
SCALE = 1.0 / math.sqrt(DH)
EXP_BIAS = -2.5
X_SCALE = 16.0
W_SCALE = 512.0
PSUM_SCALE = X_SCALE * W_SCALE  # 8192: q/k/v (and out) psum scale
V_KEEP = X_SCALE / PSUM_SCALE  # vp kept at 16*v so avn lands at 16*avn
N_CORES = 8

LAST_EXEC_NS = None
LAST_RESULTS = None

_PROGRAM = None

_DONE = object()


class FillCursor:
    """Pulls matmul-emitting generator items with position tracking so
    the schedule can gate on known stream positions. ``limit`` caps how
    far opportunistic pulls may advance (hard data dependencies that the
    chain order alone cannot express)."""

    def __init__(self, gen):
        self.gen = gen
        self.pos = 0
        self.done = False
        self.limit = 1 << 30

    def pull(self, n):
        self.pull_to(self.pos + n)

    def pull_to(self, target):
        target = min(target, self.limit)
        while self.pos < target and not self.done:
            if next(self.gen, _DONE) is _DONE:
                self.done = True
            else:
                self.pos += 1

    def drain(self):
        self.limit = 1 << 30
        self.pull_to(1 << 30)


def _chain(*gens):
    for g in gens:
        yield from g


def _emit(tc, nc, mybir, ReduceOp, xq, xk, xv, wq01, wq23, wk, wv, wo, out):
    f32 = mybir.dt.float32
    bf16 = mybir.dt.bfloat16
    f16 = mybir.dt.float16
    f8 = mybir.dt.float8e4
    Exp = mybir.ActivationFunctionType.Exp
    DR = mybir.MatmulPerfMode.DoubleRow

    # x tensors are host-packed tile-major: [(sb j p), 2*2*QB]
    xq_r = xq[:].rearrange("(sb j p) w -> p sb j w", sb=NQB, j=NJP)
    xk_r = xk[:].rearrange("(sb j p) w -> p sb j w", sb=NQB, j=NJP)
    xv_r = xv[:].rearrange("(sb j p) w -> p sb j w", sb=NQB, j=NJP)
    out_r = out[:].rearrange("(sc p) d -> p sc d", p=128)

    with tc.tile_pool(name="persist", bufs=1) as persist, \
         tc.tile_pool(name="xstream", bufs=21) as xs_pool:
        wq01_sb = persist.tile([128, 2, DC, 2, DH], f8, tag="wq01")
        wq23_sb = persist.tile([128, DC, 2, 2 * DH], f8, tag="wq23")
        wk_sb = persist.tile([128, DC, 2, DH], f8, tag="wk")
        wv_sb = persist.tile([128, DC, 2, DH], f8, tag="wv")
        wo_sb = persist.tile([128, NH, 2, D], f8, tag="wo")
        kp = persist.tile([128, S], bf16, tag="kp")  # k_proj^T (true scale)
        qp = persist.tile([128, NH, S], bf16, tag="qp")  # q_proj^T
        vp = persist.tile([128, KC, DH], bf16, tag="vp")  # v natural, 16*v
        ebias = persist.tile([128, 1], f32, tag="ebias")  # exp bias const

        def wq_slice(h, j, w_hl):
            hh = h % 2
            if h < 2:
                return wq01_sb[:, hh, 2 * j:2 * j + 2, w_hl, :]
            return wq23_sb[:, 2 * j:2 * j + 2, w_hl, hh * DH:(hh + 1) * DH]

        def load_x(src_r, sb):
            # one tile per dc-pair carrying hi+lo: [128, 2(pair), 2(hl), QB]
            tiles = []
            for j in range(NJP):
                xt = xs_pool.tile([128, 2, 2, QB], f8, tag="xs")
                nc.sync.dma_start(out=xt, in_=src_r[:, sb, j, :])
                tiles.append(xt)
            return tiles

        with tc.tile_pool(name="big_psum", bufs=2, space="PSUM") as big_psum, \
             tc.tile_pool(name="pt_pool", bufs=16) as pt_pool, \
             tc.tile_pool(name="pt2_pool", bufs=6) as pt2_pool, \
             tc.tile_pool(name="tree_pool", bufs=5) as tree_pool, \
             tc.tile_pool(name="acc_pool", bufs=2) as acc_pool, \
             tc.tile_pool(name="avn_pool", bufs=2) as avn_pool, \
             tc.tile_pool(name="ostage", bufs=2) as ostage:

            # (weights-hl, x-hl) per split pass: hi*hi, lo*hi, hi*lo
            PASS_HL = ((0, 0), (1, 0), (0, 1))

            nc.gpsimd.memset(ebias, EXP_BIAS)

            def gen_kproj(sb, kts):
                # j-outer: consume each x tile fully as it lands; o-pool psum
                ps = o_box["pool"].tile([128, QB], f32, tag="o", name="psk")
                n = 0
                for j in range(NJP):
                    for (w_hl, x_hl) in PASS_HL:
                        for half in range(2):
                            nc.tensor.matmul(
                                ps[:, half * 256:half * 256 + 256],
                                lhsT=wk_sb[:, 2 * j:2 * j + 2, w_hl, :],
                                rhs=kts[j][:, :, x_hl,
                                           half * 256:half * 256 + 256],
                                start=(n == 0), stop=(n == 47),
                                perf_mode=DR,
                            )
                            n += 1
                            if n == 48:
                                nc.scalar.mul(kp[:, sb * QB:(sb + 1) * QB],
                                              ps, 1.0 / PSUM_SCALE)
                            yield

            def gen_vproj(sb, vts):
                # natural layout: x pairs stationary, out [kpos, dh]; j-outer
                ps = o_box["pool"].tile([128, QB], f32, tag="o", name="psv")
                n = 0
                for j in range(NJP):
                    for kb in range(4):
                        for (w_hl, x_hl) in PASS_HL:
                            nc.tensor.matmul(
                                ps[:, kb * 128:(kb + 1) * 128],
                                lhsT=vts[j][:, :, x_hl,
                                            kb * 128:(kb + 1) * 128],
                                rhs=wv_sb[:, 2 * j:2 * j + 2, w_hl, :],
                                start=(n == 0), stop=(n == 95),
                                perf_mode=DR,
                            )
                            n += 1
                            if n == 96:
                                nc.scalar.mul(
                                    vp[:, 4 * sb:4 * sb + 4, :],
                                    ps[:].rearrange("p (a b) -> p a b", a=4),
                                    V_KEEP)
                            yield

            def qproj2(sb, h0, qts, act_copy=True):
                # heads h0, h0+1 into one [128,1024] psum (one bank each)
                ps = big_psum.tile([128, 1024], f32, tag="big")
                n = 0
                for j in range(NJP):
                    for hh in range(2):
                        for (w_hl, x_hl) in PASS_HL:
                            for half in range(2):
                                off = hh * 512 + half * 256
                                nc.tensor.matmul(
                                    ps[:, off:off + 256],
                                    lhsT=wq_slice(h0 + hh, j, w_hl),
                                    rhs=qts[j][:, :, x_hl,
                                               half * 256:half * 256 + 256],
                                    start=(n == 6 * hh), stop=(n == 89 + 6 * hh),
                                    perf_mode=DR,
                                )
                                n += 1
                dst = qp[:, h0:h0 + 2, sb * QB:(sb + 1) * QB]
                srcv = ps[:].rearrange("p (a b) -> p a b", a=2)
                if act_copy:
                    nc.scalar.mul(dst, srcv, 1.0 / PSUM_SCALE)
                else:
                    nc.vector.tensor_scalar_mul(dst, srcv, 1.0 / PSUM_SCALE)

            def gen_qproj_opsum(sb, h0, qts):
                # q-projection as a filler stream on the o-psum pool
                for hh in range(2):
                    h = h0 + hh
                    o = o_box["pool"].tile([128, QB], f32, tag="o", name="oq")
                    n = 0
                    for j in range(NJP):
                        for (w_hl, x_hl) in PASS_HL:
                            for half in range(2):
                                nc.tensor.matmul(
                                    o[:, half * 256:half * 256 + 256],
                                    lhsT=wq_slice(h, j, w_hl),
                                    rhs=qts[j][:, :, x_hl,
                                               half * 256:half * 256 + 256],
                                    start=(n == 0), stop=(n == 47),
                                    perf_mode=DR,
                                )
                                n += 1
                                if n == 48:
                                    nc.vector.tensor_scalar_mul(
                                        qp[:, h, sb * QB:(sb + 1) * QB], o,
                                        1.0 / PSUM_SCALE)
                                yield

            avns = [None] * NQB  # (avn_h, avn_l) per q-block

            def new_avns(qb):
                avns[qb] = (
                    avn_pool.tile([128, NH, QB], f8, tag="avh", name="avh"),
                    avn_pool.tile([128, NH, QB], f8, tag="avl", name="avl"))

            def score_pair(h, qb, kcp, state, pool=None):
                # scores+exp for k-chunk pair kcp; DVE running sums
                ss = big_psum.tile([128, 1024], f32, tag="big")
                for i in range(2):
                    kc = 2 * kcp + i
                    nc.tensor.matmul(
                        ss[:, i * 512:(i + 1) * 512],
                        lhsT=kp[:, kc * 128:(kc + 1) * 128],
                        rhs=qp[:, h, qb * QB:(qb + 1) * QB],
                        start=True, stop=True,
                    )
                pt = (pool or pt2_pool).tile([128, 2, QB], bf16, tag="pt",
                                             name="pt")
                nc.scalar.activation(
                    pt[:].rearrange("p a b -> p (a b)"), ss, Exp,
                    scale=SCALE, bias=ebias[:, 0:1])
                state["pts"].append(pt)
                if kcp % 2 == 1:
                    t = tree_pool.tile([128, 2, QB], f16, tag="t1")
                    nc.vector.tensor_add(t, state["pts"][kcp - 1],
                                         state["pts"][kcp])
                    if state["run"] is None:
                        state["run"] = t
                    else:
                        r_new = tree_pool.tile([128, 2, QB], f16, tag="t2")
                        nc.vector.tensor_add(r_new, state["run"], t)
                        state["run"] = r_new

            def av_pair(kcp, state):
                av, pt = state["av"], state["pts"][kcp]
                for i in range(2):
                    kc = 2 * kcp + i
                    nc.tensor.matmul(
                        av, lhsT=vp[:, kc, :], rhs=pt[:, i, :],
                        start=(kc == 0), stop=(kc == KC - 1),
                    )

            def finish_duo(hA, hB, qb, sts):
                # interleaved two-head finish: h_A's Pool reduce overlaps
                # h_B's DVE sums, etc.
                sf = {}
                RRs = {}
                recs = {}
                tmps = {}
                for h in (hA, hB):
                    st = sts[h]
                    sf[h] = tree_pool.tile([128, QB], f16, tag="tf",
                                           name="tf")
                    nc.vector.tensor_add(sf[h], st["run"][:, 0, :],
                                         st["run"][:, 1, :])
                for h in (hA, hB):
                    RRs[h] = acc_pool.tile([128, QB], f32, tag="RR",
                                           name="RR")
                    nc.gpsimd.partition_all_reduce(
                        RRs[h], sf[h], channels=128, reduce_op=ReduceOp.add)
                for h in (hA, hB):
                    recs[h] = acc_pool.tile([128, QB], f32, tag="rec",
                                            name="rec")
                    nc.vector.reciprocal(recs[h], RRs[h])
                for h in (hA, hB):
                    tmps[h] = acc_pool.tile([128, QB], f32, tag="tmp",
                                            name="tmp")
                    nc.vector.tensor_mul(tmps[h], sts[h]["av"], recs[h])
                avn_h, avn_l = avns[qb]
                for h in (hA, hB):
                    nc.vector.tensor_copy(avn_h[:, h, :], tmps[h])
                for h in (hA, hB):
                    nc.vector.tensor_sub(avn_l[:, h, :], tmps[h],
                                         avn_h[:, h, :])

            def finish_head(h, qb, state):
                # softmax denominators + avn hi/lo, entirely on DVE/Pool
                av = state["av"]
                sfin = tree_pool.tile([128, QB], f16, tag="tf")
                nc.vector.tensor_add(sfin, state["run"][:, 0, :],
                                     state["run"][:, 1, :])
                RR = acc_pool.tile([128, QB], f32, tag="RR")
                nc.gpsimd.partition_all_reduce(
                    RR, sfin, channels=128, reduce_op=ReduceOp.add)
                rec = acc_pool.tile([128, QB], f32, tag="rec")
                nc.vector.reciprocal(rec, RR)
                tmp = acc_pool.tile([128, QB], f32, tag="tmp")
                nc.vector.tensor_mul(tmp, av, rec)
                avn_h, avn_l = avns[qb]
                nc.vector.tensor_copy(avn_h[:, h, :], tmp)
                nc.vector.tensor_sub(avn_l[:, h, :], tmp, avn_h[:, h, :])

            def gen_av_seg(states, kcp_lo, kcp_hi, last=False):
                # AV filler for prescored heads 0/1, kcp in [lo, hi)
                for kcp in range(kcp_lo, kcp_hi):
                    for h in (0, 1):
                        st = states[h]
                        for i in range(2):
                            kc = 2 * kcp + i
                            nc.tensor.matmul(
                                st["av"], lhsT=vp[:, kc, :],
                                rhs=st["pts"][kcp][:, i, :],
                                start=(kc == 0), stop=(kc == KC - 1),
                            )
                            if (last and kcp == kcp_hi - 1 and h == 1
                                    and i == 1):
                                finish_head(0, 0, states[0])
                                finish_head(1, 0, states[1])
                            yield

            def run_duo(qb, hA, hB, fill, av_gate=None, p_ss=4, p_av=2):
                # lag-2 interleave of two heads; filler mms cover exp waits
                sts = {}
                for h in (hA, hB):
                    st = {"pts": [], "run": None}
                    st["av"] = av_box["pool"].tile([128, QB], f32, tag="av",
                                                   name="av")
                    sts[h] = st

                def do_av(kcp):
                    if av_gate is not None:
                        fill.pull_to(av_gate[kcp])
                    av_pair(kcp, sts[hA])
                    fill.pull(p_av)
                    av_pair(kcp, sts[hB])
                    fill.pull(p_av)

                for kcp in range(NKP):
                    score_pair(hA, qb, kcp, sts[hA])
                    fill.pull(p_ss)
                    score_pair(hB, qb, kcp, sts[hB])
                    fill.pull(p_ss)
                    if kcp >= 2:
                        do_av(kcp - 2)
                for kcp in (NKP - 2, NKP - 1):
                    fill.pull(p_ss)
                    do_av(kcp)
                finish_duo(hA, hB, qb, sts)
                fill.pull(2 * p_ss)

            def gen_outproj(qb):
                # out-projection of block qb as a filler stream (o-psum)
                avn_h, avn_l = avns[qb]
                for j in range(4):
                    jsl = slice(j * 128, (j + 1) * 128)
                    sc = qb * (QB // 128) + j
                    ot = ostage.tile([128, D], bf16, tag="ot", name="ot")
                    for dbp in range(4):
                        o = o_box["pool"].tile([128, 512], f32, tag="o",
                                               name="o")
                        n = 0
                        for (a_t, w_hl) in ((avn_h, 0), (avn_h, 1),
                                            (avn_l, 0)):
                            for db01 in range(2):
                                db = dbp * 2 + db01
                                for cp in range(2):
                                    nc.tensor.matmul(
                                        o[:, db01 * 256:db01 * 256 + 256],
                                        lhsT=a_t[:, 2 * cp:2 * cp + 2, jsl],
                                        rhs=wo_sb[:, 2 * cp:2 * cp + 2, w_hl,
                                                  db * 256:(db + 1) * 256],
                                        start=(n == 0), stop=(n == 11),
                                        perf_mode=DR,
                                    )
                                    n += 1
                                    if n == 12:
                                        osl = slice(dbp * 512,
                                                    (dbp + 1) * 512)
                                        nc.vector.tensor_scalar_mul(
                                            ot[:, osl], o, 1.0 / PSUM_SCALE)
                                        if dbp == 3:
                                            nc.sync.dma_start(
                                                out=out_r[:, sc, :], in_=ot)
                                    yield

            def gen_outproj_big(qb):
                # final out-projection on the (then idle) big psum pool:
                # [128,1024] tiles hold 2 db-pairs -> 4-bank pipelining
                avn_h, avn_l = avns[qb]
                for j in range(4):
                    jsl = slice(j * 128, (j + 1) * 128)
                    sc = qb * (QB // 128) + j
                    ot = ostage.tile([128, D], bf16, tag="ot", name="ot")
                    for dbq in range(2):
                        o = big_psum.tile([128, 1024], f32, tag="big",
                                          name="obig")
                        for half in range(2):
                            dbp = dbq * 2 + half
                            n = 0
                            for cp in range(2):
                                for (a_t, w_hl) in ((avn_h, 0), (avn_h, 1),
                                                    (avn_l, 0)):
                                    for db01 in range(2):
                                        db = dbp * 2 + db01
                                        nc.tensor.matmul(
                                            o[:, half * 512 + db01 * 256:
                                              half * 512 + db01 * 256 + 256],
                                            lhsT=a_t[:, 2 * cp:2 * cp + 2,
                                                     jsl],
                                            rhs=wo_sb[:, 2 * cp:2 * cp + 2,
                                                      w_hl,
                                                      db * 256:(db + 1) * 256],
                                            start=(n == 0), stop=(n == 11),
                                            perf_mode=DR,
                                        )
                                        n += 1
                                        if n == 12 and half == 1:
                                            osl = slice(dbq * 1024,
                                                        (dbq + 1) * 1024)
                                            srcv = o[:].rearrange(
                                                "p (a b) -> p a b", a=2)
                                            dstv = ot[:, osl].rearrange(
                                                "p (a b) -> p a b", a=2)
                                            if dbq == 0:
                                                nc.scalar.mul(
                                                    dstv, srcv,
                                                    1.0 / PSUM_SCALE)
                                            else:
                                                nc.vector.tensor_scalar_mul(
                                                    dstv, srcv,
                                                    1.0 / PSUM_SCALE)
                                            nc.sync.dma_start(
                                                out=out_r[:, sc, osl],
                                                in_=ot[:, osl])
                                        yield

            av_box = {}
            o_box = {}
            with tc.tile_pool(name="av_psum", bufs=2, space="PSUM") as av_ps, \
                 tc.tile_pool(name="o_psum", bufs=2, space="PSUM") as o_ps:
                av_box["pool"] = av_ps
                o_box["pool"] = o_ps

                # ---- DMA stream order (arrival order = gate order) ----
                wq01_r = wq01[:].rearrange("(hh p) w -> p hh w", hh=2)
                nc.sync.dma_start(out=wq01_sb[:, 0], in_=wq01_r[:, 0, :])
                qt0 = xs_pool.tile([128, 2, 2, QB], f8, tag="xs", name="xt")
                nc.sync.dma_start(out=qt0, in_=xq_r[:, 0, 0, :])
                nc.sync.dma_start(out=wq01_sb[:, 1], in_=wq01_r[:, 1, :])
                qts0 = [qt0]
                for j in range(1, NJP):
                    xt = xs_pool.tile([128, 2, 2, QB], f8, tag="xs",
                                      name="xt")
                    nc.sync.dma_start(out=xt, in_=xq_r[:, 0, j, :])
                    qts0.append(xt)
                nc.sync.dma_start(out=wq23_sb, in_=wq23[:])
                nc.sync.dma_start(out=wk_sb, in_=wk[:])
                kts = [load_x(xk_r, sb) for sb in range(NQB)]
                nc.sync.dma_start(out=wv_sb, in_=wv[:])
                vts = [load_x(xv_r, sb) for sb in range(NQB)]
                qts_rest = [load_x(xq_r, 1)]
                nc.sync.dma_start(out=wo_sb, in_=wo[:])
                qts_rest += [load_x(xq_r, sb) for sb in range(2, NQB)]

                # ---- PE schedule ----
                qproj2(0, 0, qts0)
                qproj2(0, 2, qts0)

                new_avns(0)
                early = {h: {"pts": [], "run": None} for h in range(2)}
                for h in range(2):
                    early[h]["av"] = av_ps.tile([128, QB], f32, tag="av",
                                                name="eav")

                # fill chain in DMA arrival order; positions:
                # k0:48 k1:96 k2:144 k3:192 | v0:288 av01:296 v1:392 av12:400
                # v2:496 av23:504 v3:600 av34:608 av46:624 av68(+fin):640
                # qp1a:688 qp1b:736
                fill = FillCursor(_chain(
                    gen_kproj(0, kts[0]), gen_kproj(1, kts[1]),
                    gen_kproj(2, kts[2]), gen_kproj(3, kts[3]),
                    gen_vproj(0, vts[0]), gen_av_seg(early, 0, 1),
                    gen_vproj(1, vts[1]), gen_av_seg(early, 1, 2),
                    gen_vproj(2, vts[2]), gen_av_seg(early, 2, 3),
                    gen_vproj(3, vts[3]), gen_av_seg(early, 3, 4),
                    gen_av_seg(early, 4, 6),
                    gen_av_seg(early, 6, 8, last=True),
                    gen_qproj_opsum(1, 0, qts_rest[0]),
                    gen_qproj_opsum(1, 2, qts_rest[0]),
                ))
                KEND = {0: 48, 1: 96, 2: 144, 3: 192}

                # prescore qb0 heads 0/1, exp-paced, filled from the chain
                for kcp in range(NKP):
                    fill.pull_to(KEND[kcp // 2])
                    for h in range(2):
                        score_pair(h, 0, kcp, early[h], pool=pt_pool)
                        fill.pull(12)

                # qb0 h2/h3 duo; gates pull the chain so vp(sb) and the
                # early heads' remaining AV pairs land before use
                AV_GATE0 = {0: 296, 1: 296, 2: 400, 3: 400,
                            4: 504, 5: 504, 6: 640, 7: 640}
                run_duo(0, 2, 3, fill, av_gate=AV_GATE0, p_ss=6, p_av=3)
                fill.drain()

                # steady state: one global fill chain across qb1-3.
                # positions: op0:192 qp2a:288 qp2b:384 | op1:576 qp3a:672
                # qp3b:768 | op2:960
                new_avns(1)
                gfill = FillCursor(_chain(
                    gen_outproj(0),
                    gen_qproj_opsum(2, 0, qts_rest[1]),
                    gen_qproj_opsum(2, 2, qts_rest[1]),
                    gen_qproj_opsum(3, 0, qts_rest[2]),
                    gen_qproj_opsum(3, 2, qts_rest[2]),
                    gen_outproj(1),
                    gen_outproj(2),
                ))
                # during qb's duos, pulls must stop before gen_outproj(qb)
                # (its avn tiles are only complete once qb finishes)
                QGATE = {2: 384, 3: 576}
                LIMIT = {1: 576, 2: 768, 3: 1 << 30}
                for qb in range(1, NQB):
                    gfill.limit = LIMIT[qb]
                    if qb > 1:
                        new_avns(qb)
                        gfill.pull_to(QGATE[qb])
                    run_duo(qb, 0, 1, gfill, p_ss=6, p_av=3)
                    run_duo(qb, 2, 3, gfill, p_ss=6, p_av=3)
                gfill.drain()
                FillCursor(gen_outproj_big(NQB - 1)).drain()


def build_program():
    global _PROGRAM
    if _PROGRAM is not None:
        return _PROGRAM
    import concourse.tile as tile
    from concourse import bacc, mybir
    from concourse.bass_isa import ReduceOp

    f8 = mybir.dt.float8e4
    bf16 = mybir.dt.bfloat16
    nc = bacc.Bacc("TRN2", target_bir_lowering=False, debug=False)
    # x: tile-major [(sb j p), 2*2*QB]; w: sbuf layout [128, dc*2*cols]
    xq = nc.declare_dram_parameter("xq", [NQB * NJP * 128, 4 * QB], f8,
                                   isOutput=False)
    xk = nc.declare_dram_parameter("xk", [NQB * NJP * 128, 4 * QB], f8,
                                   isOutput=False)
    xv = nc.declare_dram_parameter("xv", [NQB * NJP * 128, 4 * QB], f8,
                                   isOutput=False)
    wq01 = nc.declare_dram_parameter("wq01", [2 * 128, DC * 2 * DH], f8,
                                     isOutput=False)
    wq23 = nc.declare_dram_parameter("wq23", [128, DC * 2 * 2 * DH], f8,
                                     isOutput=False)
    wk = nc.declare_dram_parameter("wk", [128, DC * 2 * DH], f8,
                                   isOutput=False)
    wv = nc.declare_dram_parameter("wv", [128, DC * 2 * DH], f8,
                                   isOutput=False)
    wo = nc.declare_dram_parameter("wo", [128, NH * 2 * D], f8,
                                   isOutput=False)
    out = nc.declare_dram_parameter("out", [S, D], bf16, isOutput=True)

    with tile.TileContext(nc) as tc:
        _emit(tc, nc, mybir, ReduceOp, xq, xk, xv, wq01, wq23, wk, wv, wo,
              out)

    nc.finalize()
    _PROGRAM = nc
    return nc


def _split8(x, scale):
    """hi+lo e4m3 pair at one shared scale, stacked on a new axis 1."""
    import ml_dtypes

    f8 = ml_dtypes.float8_e4m3
    xs = np.asarray(x, np.float32) * scale
    hi = xs.astype(f8)
    lo = (xs - hi.astype(np.float32)).astype(f8)
    return np.stack([hi, lo], axis=1)  # [d0, 2, d1]


def _pack_x(xT):
    """[D, S] f32 -> tile-major fp8 [(sb j p), 2*2*QB].

    Tile (sb, j) holds element [p, i, hl, s] = split(xT)[(2j+i)*128+p,
    hl, sb*QB+s]."""
    sp = _split8(xT, X_SCALE)  # [D, 2, S]
    t = sp.reshape(NJP, 2, 128, 2, NQB, QB).transpose(4, 0, 2, 1, 3, 5)
    return np.ascontiguousarray(t.reshape(NQB * NJP * 128, 4 * QB))


def _pack_w(w):
    """[D, C] f32 -> sbuf-layout fp8 [128, (dc 2 C)]."""
    sp = _split8(w, W_SCALE)  # [D, 2, C]
    c = sp.shape[2]
    t = sp.reshape(-1, 128, 2, c).transpose(1, 0, 2, 3)
    return np.ascontiguousarray(t.reshape(128, -1))


def make_in_maps(query, key, value, Wq, Wk, Wv, Wo):
    xTs = {}
    for b in range(2):
        xTs[b] = (
            _pack_x(np.asarray(query[b], np.float32).T),
            _pack_x(np.asarray(key[b], np.float32).T),
            _pack_x(np.asarray(value[b], np.float32).T),
        )
    in_maps = []
    for core in range(N_CORES):
        b, g = core // 4, core % 4
        xqb, xkb, xvb = xTs[b]
        Wqg = np.asarray(Wq[:, g * 512:(g + 1) * 512], np.float32)
        in_maps.append({
            "xq": xqb,
            "xk": xkb,
            "xv": xvb,
            "wq01": np.concatenate([_pack_w(Wqg[:, 0:128]),
                                    _pack_w(Wqg[:, 128:256])], axis=0),
            "wq23": _pack_w(Wqg[:, 256:512]),
            "wk": _pack_w(np.asarray(Wk[:, g * 128:(g + 1) * 128], np.float32)),
            "wv": _pack_w(np.asarray(Wv[:, g * 128:(g + 1) * 128], np.float32)),
            "wo": _pack_w(np.asarray(Wo[g * 512:(g + 1) * 512, :], np.float32)),
        })
    return in_maps


def kernel(query, key, value, mask, Wq, Wk, Wv, Wo):
    global LAST_EXEC_NS, LAST_RESULTS
    del mask  # all-ones in this problem; softmax masking is a no-op
    nc = build_program()
    in_maps = make_in_maps(query, key, value, Wq, Wk, Wv, Wo)

    from concourse.bass_utils import run_bass_kernel_spmd

    res = run_bass_kernel_spmd(nc, in_maps, core_ids=list(range(N_CORES)))
    LAST_EXEC_NS = res.exec_time_ns
    LAST_RESULTS = res
    outs = [np.asarray(r["out"], np.float32) for r in res.results]
    full = np.empty((2, S, D), np.float32)
    for b in range(2):
        full[b] = outs[b * 4] + outs[b * 4 + 1] + outs[b * 4 + 2] + outs[b * 4 + 3]
    return full
